# revision 13
# baseline (speedup 1.0000x reference)
"""Trainium2 Bass kernel for nn_BeliefStateWrapper loss_fn.

Computation (reference):
    fb = concat(forward_embeds[:, fi], backward_embeds[:, bi], -1)   [B, N, 2D]
    h  = leaky_relu(fb @ w1 + b1)                                    [B, N, D]
    logits = h @ w2 + b2                                             [B, N, 2V]
    logp = log_softmax(logits.reshape(B, N, 2, V), -1)
    labels = stack(seq[:, fi], seq[:, bi], -1)
    loss = mean(-take(logp, labels) * (1.0, 0.25))

Strategy (8 NeuronCores, SPMD — one program, per-core data):
  * Pair gather / concat / transpose is host-side input prep; the device
    receives fbT [2D, Rpad] in fp8e4 (scaled x8), R = B*N rows padded to 128.
  * w2 [D, 2V] is tensor-parallel along vocab: core c gets the fp8 slice
    w2[:, c*8000:(c+1)*8000] * 256.  Cores 0-3 cover the forward branch,
    cores 4-7 the backward branch.
  * All GEMMs run in fp8e4 with MatmulPerfMode.DoubleRow (2 k-subtiles per
    instruction) for double PE throughput (~157 TF/s streaming).  Scales keep
    every operand in the fp8 normal range: fb*8, w1*64, h*16, w2*256; the
    product scale 1/4096 is folded into the exp's activation scale.
  * Phase 1 (GEMM1 + Lrelu on the ACT engine) runs quarter-outer chasing the
    fbT DMAs; a short run of dummy PE matmuls during the DMA window holds the
    clock up and the Lrelu/Exp ACT table sets are pre-warmed to dodge
    mid-flow loads.
  * hT is DMA'd to DRAM during phase 3 (idle bandwidth); the host computes
    the per-row label logits labf/labb = h . w2[:, label] directly (2606
    dot-512s), so no on-device label GEMM is needed.
  * Phase 3 (the big vocab GEMM) uses a 4-deep rotation of [128,1024] psum
    tiles, two per 2048-col vocab block: the ACT engine exp+accumulates the
    first 1024 cols while the DVE applies a mean-corrected Schraudolph
    bit-trick exp (int16/bf16) to the rest.  Separate tiles keep the two
    consumers unserialized, and each consumer's per-tile latency is below
    the PE's production time, so the PE streams at its floor.  The DVE's
    PWL-exp slices ride otherwise-idle DMA to DRAM per half-chunk; the host
    sums them.
  * Host combine: lse = log(sum of partials), nll = lse - label_logit,
    weighted mean.  (b2 is asserted zero, as constructed.)
"""

import numpy as np

import concourse.bass as bass
import concourse.bacc as bacc
import concourse.mybir as mybir
import concourse.tile as tile
from concourse import bass_utils

P = 128          # SBUF partitions
D = 512          # hidden dim
E = 1024         # 2*D, GEMM1 contraction
NCORES = 8

_DC = D // P     # 4 d-chunks
_EO = E // P     # 8 e-chunks

FB_SCALE = 8.0
W1_SCALE = 64.0
H_SCALE = 16.0
W2_SCALE = 256.0
LOGIT_SCALE = H_SCALE * W2_SCALE          # psum2 = 4096 * logit
H_PS_SCALE = H_SCALE / (FB_SCALE * W1_SCALE)   # psum1 * this = 16*h_pre


# Schraudolph fast-exp constants (DVE path), bf16 flavor:
#   i16 = psum * K1 + K2 ; bitcast i16 -> bf16 ~= exp(psum / LOGIT_SCALE)
# mean multiplicative PWL error E[(1+f)/2^f] is divided out via the offset.
_LOG2E = 1.4426950408889634
_F = np.linspace(0.0, 1.0, 1 << 16, endpoint=False) + 0.5 / (1 << 16)
_I = float(np.mean((1.0 + _F) * np.exp2(-_F)))
_C_SHIFT = float(np.log2(_I))            # subtract so mean error == 1
K1 = _LOG2E * 128.0 / LOGIT_SCALE
# +0.25: split the difference between truncating and rounding f32->i16
K2 = (127.0 - _C_SHIFT) * 128.0 + 0.25

_nc_cache = {}


def build_program(rpad: int, vs: int):
    """Build the SPMD Bass program (same NEFF for all 8 cores).

    rpad: padded row count (multiple of 128)
    vs:   per-core vocab slice width (2V / 8 = 8000)
    """
    nch = rpad // P                  # row chunks (21)
    f32 = mybir.dt.float32
    fp8 = mybir.dt.float8e4
    i16 = mybir.dt.int16
    DR = mybir.MatmulPerfMode.DoubleRow

    nc = bacc.Bacc("TRN2", target_bir_lowering=False, debug=False,
                   enable_asserts=False)

    fbt_d = nc.dram_tensor("fbt", [E, rpad], fp8, kind="ExternalInput").ap()
    w1_d = nc.dram_tensor("w1", [E, D], fp8, kind="ExternalInput").ap()
    b1_d = nc.dram_tensor("b1", [D], f32, kind="ExternalInput").ap()
    w2s_d = nc.dram_tensor("w2s", [D, vs], fp8, kind="ExternalInput").ap()

    se_d = nc.dram_tensor("se", [P, nch * 4], f32, kind="ExternalOutput").ap()
    ht_d = nc.dram_tensor("ht", [P, _DC, rpad], fp8, kind="ExternalOutput").ap()
    # per-et-block split: first 1024 cols -> ACT exp, rest -> DVE fast-exp
    dvws = [1024, 1024, 1024, vs - 3 * 2048 - 1024]      # DVE widths (832 last)
    dvtot = sum(dvws)                                    # 3904
    dvs_d = nc.dram_tensor("dvs", [P, nch * dvtot], i16,
                           kind="ExternalOutput").ap()

    with tile.TileContext(nc, pool_alloc_mode="queue") as tc:
        with (
            tc.tile_pool(name="pers", bufs=1) as pers,
            tc.tile_pool(name="psum", bufs=4, space="PSUM") as psum,
            tc.tile_pool(name="scratch", bufs=3) as scratch,
        ):
            # ---- resident tensors -------------------------------------
            # Issue order per queue engine = program order; transfers are
            # FIFO per queue, spread over the Sync and Activation HWDGE
            # queues so the two streams run in parallel.  Needed-by times:
            # w1+fbt q0 ~11us (GEMM1 start), q1 ~17us, q2 ~23us, w2s full
            # by ~29.5us (phase-3 start).
            # b1 first on the scalar queue: it is tiny (2KB) and phase 1's
            # first Lrelu blocks on it, so it must not sit behind the bulk
            b1_t = pers.tile([P, _DC], f32, tag="b1")
            nc.scalar.dma_start(out=b1_t[:],
                                in_=b1_d.rearrange("(dc p) -> p dc", p=P))

            # w1 split across both queues so GEMM1's stationary lands ~11us
            w1_t = pers.tile([P, _EO, D], fp8, tag="w1")
            w1_r = w1_d.rearrange("(eo p) d -> p eo d", p=P)
            nc.sync.dma_start(out=w1_t[:, :4], in_=w1_r[:, :4])
            nc.scalar.dma_start(out=w1_t[:, 4:], in_=w1_r[:, 4:])

            fbt_t = pers.tile([P, _EO, rpad], fp8, tag="fbt")
            w2s_t = pers.tile([P, _DC, vs], fp8, tag="w2s")
            # first two pieces are 512 rows so GEMM1 can start ~2.7us
            # earlier (it only needs w1 + the first 512-row slab)
            qrows = [(0, 512), (512, 512)]
            q0 = 1024
            while q0 < rpad:
                qw = min(1024, rpad - q0)
                qrows.append((q0, qw))
                q0 += qw

            fbt_r = fbt_d.rearrange("(eo p) r -> p eo r", p=P)
            w2s_r = w2s_d.rearrange("(dc p) v -> p dc v", p=P)

            # all of fbt ahead of w2s: GEMM1 consumes quarters starting
            # ~11us and w2s is not needed until phase 3 (~29us)
            for (q0, qw) in qrows:
                for hh in range(4):
                    eng = nc.sync if hh % 2 == 0 else nc.scalar
                    eng.dma_start(
                        out=fbt_t[:, hh * 2:(hh + 1) * 2, q0:q0 + qw],
                        in_=fbt_r[:, hh * 2:(hh + 1) * 2, q0:q0 + qw])

            hT = pers.tile([P, _DC, rpad], fp8, tag="hT")
            se_t = pers.tile([P, nch * 4], f32, tag="se")
            nc.gpsimd.memset(se_t[:], 0.0)

            # ACT table loads fire on each func/config transition in the ACT
            # instruction stream, so keep it to two runs: one Lrelu warm
            # (signature-matched: vector bias, same scale/alpha/out-dtype)
            # ahead of all the phase-1 Lrelus, and one Exp warm after them
            # (its load hides under phase 3's first matmuls).
            warm = pers.tile([P, 8], f32, tag="warm")
            warmo = pers.tile([P, 8], fp8, tag="warmo")
            warme = pers.tile([P, 8], mybir.dt.bfloat16, tag="warme")
            nc.gpsimd.memset(warm[:], 0.0)
            nc.scalar.activation(out=warmo[:], in_=warm[:],
                                 func=mybir.ActivationFunctionType.Lrelu,
                                 scale=H_PS_SCALE, bias=b1_t[:, 0:1],
                                 alpha=0.01)

            for hh in range(2):
                half = vs // 2
                eng = nc.sync if hh == 0 else nc.scalar
                eng.dma_start(
                    out=w2s_t[:, :, hh * half:(hh + 1) * half],
                    in_=w2s_r[:, :, hh * half:(hh + 1) * half])

            # hold the PE clock up (DVFS ramps with sustained use) with a
            # short run of dummy matmuls while the input DMAs stream in;
            # GEMM1's first rows land ~11us, so ~8 dummies suffice.
            wmm = pers.tile([P, 2, 512], fp8, tag="wmm")
            nc.gpsimd.memset(wmm[:], 0.0)
            wps = psum.tile([P, 1024], f32, tag="ps", name="warmps")
            for _ in range(9):
                nc.tensor.matmul(wps[:, :512], lhsT=wmm[:, :, :P],
                                 rhs=wmm[:], start=True, stop=True,
                                 perf_mode=DR)

            # ---- phase 1: hT = 16 * leaky_relu(w1.T @ fbT + b1) --------
            # halves of <=1024 rows; within a half the stationary w1 slice
            # is reused across all row subgroups so LDWEIGHTS stays hidden.
            halves = list(qrows)

            for qi, (h0, hw_) in enumerate(halves):
                for dc in range(_DC):
                    ps = psum.tile([P, 1024], f32, tag="ps")
                    for e2 in range(_EO // 2):
                        sg0 = 0
                        while sg0 < hw_:
                            g = min(512, hw_ - sg0)
                            nc.tensor.matmul(
                                ps[:, sg0:sg0 + g],
                                lhsT=w1_t[:, 2 * e2:2 * e2 + 2, dc * P:(dc + 1) * P],
                                rhs=fbt_t[:, 2 * e2:2 * e2 + 2, h0 + sg0:h0 + sg0 + g],
                                start=(e2 == 0),
                                stop=(e2 == _EO // 2 - 1),
                                perf_mode=DR,
                            )
                            sg0 += g
                    # leaky on the (otherwise idle here) ACT engine:
                    # Lrelu(ps*scale + b1*16) with negative slope alpha
                    nc.scalar.activation(
                        out=hT[:, dc, h0:h0 + hw_], in_=ps[:, :hw_],
                        func=mybir.ActivationFunctionType.Lrelu,
                        scale=H_PS_SCALE, bias=b1_t[:, dc:dc + 1], alpha=0.01)
            # Exp table warm: the load (~1.3us) hides under phase 3's first
            # chunk of matmuls before the first real exp needs it
            nc.scalar.activation(out=warme[:], in_=warm[:],
                                 func=mybir.ActivationFunctionType.Exp,
                                 scale=1.0 / LOGIT_SCALE,
                                 accum_out=se_t[:, 0:1])
            # ship hT to the host (label logits are host-side dot products)
            # on the otherwise-idle gpsimd SWDGE queue
            nc.gpsimd.dma_start(out=ht_d, in_=hT[:])

            # ---- phase 3 per row chunk: vocab GEMM + exp + row-sum -----
            # Every psum tile is consumed by BOTH engines in parallel: ACT
            # exp+accumulate on the first aw cols, DVE Schraudolph fast-exp
            # on the last DVW cols.  Each engine's per-tile latency stays
            # below the PE's per-tile production time, so the 2-slot psum
            # rotation never stalls the PE.  The DVE's int16 PWL-exp slices
            # ride an (idle) DMA to DRAM per half-chunk; the host sums them.
            dvo_of = [0, dvws[0], dvws[0] + dvws[1], dvws[0] + dvws[1] + dvws[2]]
            for k in range(nch):
                dv = scratch.tile([P, dvtot], i16, tag="dv", name=f"dv{k}")
                # et blocks processed in pairs: 4 live psum tiles (8 banks),
                # dc2-outer within the pair so each hT stationary serves 8
                # matmuls, and consumers get a full pair period to drain
                for pr in range(2):
                    tiles = []
                    for et in (2 * pr, 2 * pr + 1):
                        psa = psum.tile([P, 1024], f32, tag="ps",
                                        name=f"a{k}_{et}")
                        psd = psum.tile([P, 1024], f32, tag="ps",
                                        name=f"d{k}_{et}")
                        tiles.append((psa, psd))
                    for dc2 in range(2):
                        for ei, et in enumerate((2 * pr, 2 * pr + 1)):
                            psa, psd = tiles[ei]
                            for sub in range(2 + (dvws[et] + 511) // 512):
                                vb = et * 2048 + sub * 512
                                nw = min(512, vs - vb)
                                dst = (psa[:, sub * 512:sub * 512 + nw]
                                       if sub < 2 else
                                       psd[:, (sub - 2) * 512:
                                              (sub - 2) * 512 + nw])
                                nc.tensor.matmul(
                                    dst,
                                    lhsT=hT[:, 2 * dc2:2 * dc2 + 2,
                                            k * P:(k + 1) * P],
                                    rhs=w2s_t[:, 2 * dc2:2 * dc2 + 2,
                                              vb:vb + nw],
                                    start=(dc2 == 0),
                                    stop=(dc2 == 1),
                                    perf_mode=DR,
                                )
                    for ei, et in enumerate((2 * pr, 2 * pr + 1)):
                        psa, psd = tiles[ei]
                        dw = dvws[et]
                        ej = scratch.tile([P, 1024], mybir.dt.bfloat16,
                                          tag="ej", name=f"ej{k}_{et}")
                        nc.scalar.activation(
                            out=ej[:], in_=psa[:],
                            func=mybir.ActivationFunctionType.Exp,
                            scale=1.0 / LOGIT_SCALE,
                            accum_out=se_t[:, k * 4 + et: k * 4 + et + 1])
                        nc.vector.tensor_scalar(
                            dv[:, dvo_of[et]:dvo_of[et] + dw], psd[:, :dw],
                            K1, K2,
                            mybir.AluOpType.mult, mybir.AluOpType.add)
                    # flush this half-chunk's DVE slice so the final DMA
                    # at kernel end is small
                    lo = dvo_of[2 * pr]
                    hi = dvo_of[2 * pr + 1] + dvws[2 * pr + 1]
                    if k == nch - 1 and pr == 1:
                        nc.sync.dma_start(
                            out=dvs_d[:, k * dvtot + lo:k * dvtot + dvo_of[3]],
                            in_=dv[:, lo:dvo_of[3]])
                        nc.sync.dma_start(
                            out=dvs_d[:, k * dvtot + dvo_of[3]:(k + 1) * dvtot],
                            in_=dv[:, dvo_of[3]:])
                    else:
                        nc.sync.dma_start(
                            out=dvs_d[:, k * dvtot + lo:k * dvtot + hi],
                            in_=dv[:, lo:hi])

            # ---- outputs ----------------------------------------------
            nc.sync.dma_start(out=se_d[:], in_=se_t[:])

    nc.compile()
    return nc


def _prep_inputs(forward_embeds, backward_embeds, seq, fi, bi, w1, b1, w2, b2):
    import ml_dtypes
    fp8 = ml_dtypes.float8_e4m3

    fwd = np.asarray(forward_embeds, np.float32)
    bwd = np.asarray(backward_embeds, np.float32)
    seq = np.asarray(seq)
    fi = np.asarray(fi).astype(np.int64)
    bi = np.asarray(bi).astype(np.int64)
    w1 = np.asarray(w1, np.float32)
    b1 = np.asarray(b1, np.float32)
    w2 = np.asarray(w2, np.float32)
    b2 = np.asarray(b2, np.float32)

    B, L, Dd = fwd.shape
    assert Dd == D
    N = fi.shape[0]
    V = w2.shape[1] // 2
    R = B * N
    nch = (R + P - 1) // P
    rpad = nch * P
    vs = (2 * V) // NCORES

    assert not np.any(b2), "kernel assumes b2 == 0 (as in setup_inputs)"

    def q8(x):
        return np.clip(x, -240.0, 240.0).astype(fp8)

    # host-side gather + transpose (the sharding/layout prep)
    fb = np.concatenate([fwd[:, fi, :], bwd[:, bi, :]], axis=-1)  # [B, N, 2D]
    fb = fb.reshape(R, E)
    fbT = np.zeros((E, rpad), dtype=fp8)
    fbT[:, :R] = q8(fb.T * FB_SCALE)

    labels_f = seq[np.arange(B)[:, None], fi[None, :]].reshape(R).astype(np.int64)
    labels_b = seq[np.arange(B)[:, None], bi[None, :]].reshape(R).astype(np.int64)

    w1q = q8(w1 * W1_SCALE)
    b1s = (b1 * H_SCALE).astype(np.float32)

    shared = dict(fbt=fbT, w1=w1q, b1=b1s)
    in_maps = []
    for c in range(NCORES):
        m = dict(shared)
        m["w2s"] = q8(np.ascontiguousarray(w2[:, c * vs:(c + 1) * vs]) * W2_SCALE)
        in_maps.append(m)

    meta = dict(B=B, N=N, V=V, R=R, nch=nch, rpad=rpad, vs=vs,
                labels_f=labels_f, labels_b=labels_b, w2=w2)
    return in_maps, meta


def _combine(results, meta):
    import ml_dtypes
    R, nch, V = meta["R"], meta["nch"], meta["V"]
    # per-core partial sums of exp(logit) over its vocab slice
    S = []
    for c in range(NCORES):
        se = np.asarray(results[c]["se"], np.float64)          # [128, nch*4]
        s = se.reshape(P, nch, 4).sum(-1)                      # [128, nch]
        dvs = np.asarray(results[c]["dvs"])                    # [128, nch*3904]
        ex = dvs.view(ml_dtypes.bfloat16).astype(np.float32)
        s = s + ex.reshape(P, nch, -1).sum(-1)
        S.append(s.T.reshape(-1)[:R])                          # row-major [R]
    Sf = S[0] + S[1] + S[2] + S[3]
    Sb = S[4] + S[5] + S[6] + S[7]

    # label logits: host-side dot products against the exact (f32) w2
    # columns, using the device's fp8 h (ht = 16*h as fp8)
    ht = np.asarray(results[0]["ht"]).astype(np.float32)       # [128, 4, rpad]
    hv = ht.transpose(2, 1, 0).reshape(-1, D)[:R]              # [R, 512] = 16*h
    w2 = meta["w2"]
    labf = np.einsum('rd,dr->r', hv, w2[:, meta["labels_f"]],
                     dtype=np.float64) / H_SCALE
    labb = np.einsum('rd,dr->r', hv, w2[:, V + meta["labels_b"]],
                     dtype=np.float64) / H_SCALE

    nll_f = np.log(Sf) - labf
    nll_b = np.log(Sb) - labb
    loss = (1.0 * nll_f + 0.25 * nll_b).sum() / (R * 2)
    return np.float32(loss)


def kernel(**inputs) -> np.ndarray:
    in_maps, meta = _prep_inputs(**inputs)

    key = (meta["rpad"], meta["vs"])
    if key not in _nc_cache:
        _nc_cache[key] = build_program(*key)
    nc = _nc_cache[key]

    res = bass_utils.run_bass_kernel_spmd(nc, in_maps, core_ids=list(range(NCORES)))
    return _combine(res.results, meta)


if __name__ == "__main__":
    import reference
    ins = reference.setup_inputs()
    expected = np.asarray(reference.reference(**ins))
    actual = kernel(**{k: np.asarray(v) for k, v in ins.items()})
    rel = abs(float(actual) - float(expected)) / max(abs(float(expected)), 1e-9)
    print(f"expected {float(expected):.6f}  actual {float(actual):.6f}  rel {rel:.3e}")


# revision 16
# speedup vs baseline: 1.0258x; 1.0258x over previous
"""Trainium2 Bass kernel for nn_BeliefStateWrapper loss_fn.

Computation (reference):
    fb = concat(forward_embeds[:, fi], backward_embeds[:, bi], -1)   [B, N, 2D]
    h  = leaky_relu(fb @ w1 + b1)                                    [B, N, D]
    logits = h @ w2 + b2                                             [B, N, 2V]
    logp = log_softmax(logits.reshape(B, N, 2, V), -1)
    labels = stack(seq[:, fi], seq[:, bi], -1)
    loss = mean(-take(logp, labels) * (1.0, 0.25))

Strategy (8 NeuronCores, SPMD — one program, per-core data):
  * Pair gather / concat / transpose is host-side input prep; the device
    receives fbT [2D, Rpad] in fp8e4 (scaled x8), R = B*N rows padded to 128.
  * w2 [D, 2V] is tensor-parallel along vocab: core c gets the fp8 slice
    w2[:, c*8000:(c+1)*8000] * 256.  Cores 0-3 cover the forward branch,
    cores 4-7 the backward branch.
  * All GEMMs run in fp8e4 with MatmulPerfMode.DoubleRow (2 k-subtiles per
    instruction) for double PE throughput (~157 TF/s streaming).  Scales keep
    every operand in the fp8 normal range: fb*8, w1*64, h*16, w2*256; the
    product scale 1/4096 is folded into the exp's activation scale.
  * Phase 1 (GEMM1 + Lrelu on the ACT engine) runs quarter-outer chasing the
    fbT DMAs; a short run of dummy PE matmuls during the DMA window holds the
    clock up and the Lrelu/Exp ACT table sets are pre-warmed to dodge
    mid-flow loads.
  * hT is DMA'd to DRAM during phase 3 (idle bandwidth); the host computes
    the per-row label logits labf/labb = h . w2[:, label] directly (2606
    dot-512s), so no on-device label GEMM is needed.
  * Phase 3 (the big vocab GEMM) uses a 4-deep rotation of [128,1024] psum
    tiles, two per 2048-col vocab block: the ACT engine exp+accumulates the
    first 1024 cols while the DVE applies a mean-corrected Schraudolph
    bit-trick exp (int16/bf16) to the rest.  Separate tiles keep the two
    consumers unserialized, and each consumer's per-tile latency is below
    the PE's production time, so the PE streams at its floor.  The DVE's
    PWL-exp slices ride otherwise-idle DMA to DRAM per half-chunk; the host
    sums them.
  * Host combine: lse = log(sum of partials), nll = lse - label_logit,
    weighted mean.  (b2 is asserted zero, as constructed.)
"""

import numpy as np

import concourse.bass as bass
import concourse.bacc as bacc
import concourse.mybir as mybir
import concourse.tile as tile
from concourse import bass_utils

P = 128          # SBUF partitions
D = 512          # hidden dim
E = 1024         # 2*D, GEMM1 contraction
NCORES = 8

_DC = D // P     # 4 d-chunks
_EO = E // P     # 8 e-chunks

FB_SCALE = 8.0
W1_SCALE = 64.0
H_SCALE = 16.0
W2_SCALE = 256.0
LOGIT_SCALE = H_SCALE * W2_SCALE          # psum2 = 4096 * logit
H_PS_SCALE = H_SCALE / (FB_SCALE * W1_SCALE)   # psum1 * this = 16*h_pre


# Schraudolph fast-exp constants (DVE path), bf16 flavor:
#   i16 = psum * K1 + K2 ; bitcast i16 -> bf16 ~= exp(psum / LOGIT_SCALE)
# mean multiplicative PWL error E[(1+f)/2^f] is divided out via the offset.
_LOG2E = 1.4426950408889634
_F = np.linspace(0.0, 1.0, 1 << 16, endpoint=False) + 0.5 / (1 << 16)
_I = float(np.mean((1.0 + _F) * np.exp2(-_F)))
_C_SHIFT = float(np.log2(_I))            # subtract so mean error == 1
K1 = _LOG2E * 128.0 / LOGIT_SCALE
# +0.25: split the difference between truncating and rounding f32->i16
K2 = (127.0 - _C_SHIFT) * 128.0 + 0.25

_nc_cache = {}


def build_program(rpad: int, vs: int):
    """Build the SPMD Bass program (same NEFF for all 8 cores).

    rpad: padded row count (multiple of 128)
    vs:   per-core vocab slice width (2V / 8 = 8000)
    """
    nch = rpad // P                  # row chunks (21)
    f32 = mybir.dt.float32
    fp8 = mybir.dt.float8e4
    i16 = mybir.dt.int16
    DR = mybir.MatmulPerfMode.DoubleRow

    nc = bacc.Bacc("TRN2", target_bir_lowering=False, debug=False,
                   enable_asserts=False)

    fbt_d = nc.dram_tensor("fbt", [E, rpad], fp8, kind="ExternalInput").ap()
    w1_d = nc.dram_tensor("w1", [E, D], fp8, kind="ExternalInput").ap()
    b1_d = nc.dram_tensor("b1", [D], f32, kind="ExternalInput").ap()
    w2s_d = nc.dram_tensor("w2s", [D, vs], fp8, kind="ExternalInput").ap()

    se_d = nc.dram_tensor("se", [P, nch * 4], f32, kind="ExternalOutput").ap()
    ht_d = nc.dram_tensor("ht", [P, _DC, rpad], fp8, kind="ExternalOutput").ap()
    # per-et-block split: first 1024 cols -> ACT exp, rest -> DVE fast-exp
    dvws = [1024, 1024, 1024, vs - 3 * 2048 - 1024]      # DVE widths (832 last)
    dvtot = sum(dvws)                                    # 3904
    dvs_d = nc.dram_tensor("dvs", [P, nch * dvtot], i16,
                           kind="ExternalOutput").ap()

    with tile.TileContext(nc, pool_alloc_mode="queue") as tc:
        with (
            tc.tile_pool(name="pers", bufs=1) as pers,
            tc.tile_pool(name="psum", bufs=4, space="PSUM") as psum,
            tc.tile_pool(name="scratch", bufs=3) as scratch,
        ):
            # ---- resident tensors -------------------------------------
            # Issue order per queue engine = program order; transfers are
            # FIFO per queue, spread over the Sync and Activation HWDGE
            # queues so the two streams run in parallel.  Needed-by times:
            # w1+fbt q0 ~11us (GEMM1 start), q1 ~17us, q2 ~23us, w2s full
            # by ~29.5us (phase-3 start).
            # b1 first on the scalar queue: it is tiny (2KB) and phase 1's
            # first Lrelu blocks on it, so it must not sit behind the bulk
            b1_t = pers.tile([P, _DC], f32, tag="b1")
            nc.scalar.dma_start(out=b1_t[:],
                                in_=b1_d.rearrange("(dc p) -> p dc", p=P))

            # w1 split across both queues so GEMM1's stationary lands ~11us
            w1_t = pers.tile([P, _EO, D], fp8, tag="w1")
            w1_r = w1_d.rearrange("(eo p) d -> p eo d", p=P)
            nc.sync.dma_start(out=w1_t[:, :4], in_=w1_r[:, :4])
            nc.scalar.dma_start(out=w1_t[:, 4:], in_=w1_r[:, 4:])

            fbt_t = pers.tile([P, _EO, rpad], fp8, tag="fbt")
            w2s_t = pers.tile([P, _DC, vs], fp8, tag="w2s")
            qrows = []
            q0 = 0
            while q0 < rpad:
                qw = min(1024, rpad - q0)
                qrows.append((q0, qw))
                q0 += qw

            fbt_r = fbt_d.rearrange("(eo p) r -> p eo r", p=P)
            w2s_r = w2s_d.rearrange("(dc p) v -> p dc v", p=P)

            # all of fbt ahead of w2s: GEMM1 consumes quarters starting
            # ~11us and w2s is not needed until phase 3 (~29us)
            for (q0, qw) in qrows:
                for hh in range(4):
                    eng = nc.sync if hh % 2 == 0 else nc.scalar
                    eng.dma_start(
                        out=fbt_t[:, hh * 2:(hh + 1) * 2, q0:q0 + qw],
                        in_=fbt_r[:, hh * 2:(hh + 1) * 2, q0:q0 + qw])

            hT = pers.tile([P, _DC, rpad], fp8, tag="hT")
            se_t = pers.tile([P, nch * 4], f32, tag="se")
            nc.gpsimd.memset(se_t[:], 0.0)

            # ACT table loads fire on each func/config transition in the ACT
            # instruction stream, so keep it to two runs: one Lrelu warm
            # (signature-matched: vector bias, same scale/alpha/out-dtype)
            # ahead of all the phase-1 Lrelus, and one Exp warm after them
            # (its load hides under phase 3's first matmuls).
            warm = pers.tile([P, 8], f32, tag="warm")
            warmo = pers.tile([P, 8], fp8, tag="warmo")
            warme = pers.tile([P, 8], mybir.dt.bfloat16, tag="warme")
            nc.gpsimd.memset(warm[:], 0.0)
            nc.scalar.activation(out=warmo[:], in_=warm[:],
                                 func=mybir.ActivationFunctionType.Lrelu,
                                 scale=H_PS_SCALE, bias=b1_t[:, 0:1],
                                 alpha=0.01)

            for hh in range(2):
                half = vs // 2
                eng = nc.sync if hh == 0 else nc.scalar
                eng.dma_start(
                    out=w2s_t[:, :, hh * half:(hh + 1) * half],
                    in_=w2s_r[:, :, hh * half:(hh + 1) * half])

            # hold the PE clock up (DVFS ramps with sustained use) with a
            # short run of dummy matmuls while the input DMAs stream in;
            # GEMM1's first rows land ~11us, so ~8 dummies suffice.
            wmm = pers.tile([P, 2, 512], fp8, tag="wmm")
            nc.gpsimd.memset(wmm[:], 0.0)
            wps = psum.tile([P, 1024], f32, tag="ps", name="warmps")
            for _ in range(12):
                nc.tensor.matmul(wps[:, :512], lhsT=wmm[:, :, :P],
                                 rhs=wmm[:], start=True, stop=True,
                                 perf_mode=DR)

            # ---- phase 1: hT = 16 * leaky_relu(w1.T @ fbT + b1) --------
            # halves of <=1024 rows; within a half the stationary w1 slice
            # is reused across all row subgroups so LDWEIGHTS stays hidden.
            halves = list(qrows)

            for qi, (h0, hw_) in enumerate(halves):
                for dc in range(_DC):
                    ps = psum.tile([P, 1024], f32, tag="ps")
                    for e2 in range(_EO // 2):
                        sg0 = 0
                        while sg0 < hw_:
                            g = min(512, hw_ - sg0)
                            nc.tensor.matmul(
                                ps[:, sg0:sg0 + g],
                                lhsT=w1_t[:, 2 * e2:2 * e2 + 2, dc * P:(dc + 1) * P],
                                rhs=fbt_t[:, 2 * e2:2 * e2 + 2, h0 + sg0:h0 + sg0 + g],
                                start=(e2 == 0),
                                stop=(e2 == _EO // 2 - 1),
                                perf_mode=DR,
                            )
                            sg0 += g
                    # leaky on the (otherwise idle here) ACT engine:
                    # Lrelu(ps*scale + b1*16) with negative slope alpha
                    nc.scalar.activation(
                        out=hT[:, dc, h0:h0 + hw_], in_=ps[:, :hw_],
                        func=mybir.ActivationFunctionType.Lrelu,
                        scale=H_PS_SCALE, bias=b1_t[:, dc:dc + 1], alpha=0.01)
            # Exp table warm: the load (~1.3us) hides under phase 3's first
            # chunk of matmuls before the first real exp needs it
            nc.scalar.activation(out=warme[:], in_=warm[:],
                                 func=mybir.ActivationFunctionType.Exp,
                                 scale=1.0 / LOGIT_SCALE,
                                 accum_out=se_t[:, 0:1])
            # ship hT to the host (label logits are host-side dot products)
            # on the otherwise-idle gpsimd SWDGE queue
            nc.gpsimd.dma_start(out=ht_d, in_=hT[:])

            # ---- phase 3 per row chunk: vocab GEMM + exp + row-sum -----
            # Every psum tile is consumed by BOTH engines in parallel: ACT
            # exp+accumulate on the first aw cols, DVE Schraudolph fast-exp
            # on the last DVW cols.  Each engine's per-tile latency stays
            # below the PE's per-tile production time, so the 2-slot psum
            # rotation never stalls the PE.  The DVE's int16 PWL-exp slices
            # ride an (idle) DMA to DRAM per half-chunk; the host sums them.
            dvo_of = [0, dvws[0], dvws[0] + dvws[1], dvws[0] + dvws[1] + dvws[2]]
            for k in range(nch):
                dv = scratch.tile([P, dvtot], i16, tag="dv", name=f"dv{k}")
                # et blocks processed in pairs: 4 live psum tiles (8 banks),
                # dc2-outer within the pair so each hT stationary serves 8
                # matmuls, and consumers get a full pair period to drain.
                # The very last pair runs et3 before et2 so its wide ACT/DVE
                # consumers drain under the final matmuls (shorter tail).
                for pr in range(2):
                    ets = (2 * pr, 2 * pr + 1)
                    last_pr = (k == nch - 1 and pr == 1)
                    if last_pr:
                        ets = (3, 2)
                    tiles = {}
                    for et in ets:
                        psa = psum.tile([P, 1024], f32, tag="ps",
                                        name=f"a{k}_{et}")
                        psd = psum.tile([P, 1024], f32, tag="ps",
                                        name=f"d{k}_{et}")
                        tiles[et] = (psa, psd)
                    for dc2 in range(2):
                        for et in ets:
                            psa, psd = tiles[et]
                            for sub in range(2 + (dvws[et] + 511) // 512):
                                vb = et * 2048 + sub * 512
                                nw = min(512, vs - vb)
                                dst = (psa[:, sub * 512:sub * 512 + nw]
                                       if sub < 2 else
                                       psd[:, (sub - 2) * 512:
                                              (sub - 2) * 512 + nw])
                                nc.tensor.matmul(
                                    dst,
                                    lhsT=hT[:, 2 * dc2:2 * dc2 + 2,
                                            k * P:(k + 1) * P],
                                    rhs=w2s_t[:, 2 * dc2:2 * dc2 + 2,
                                              vb:vb + nw],
                                    start=(dc2 == 0),
                                    stop=(dc2 == 1),
                                    perf_mode=DR,
                                )
                    for et in ets:
                        psa, psd = tiles[et]
                        dw = dvws[et]
                        ej = scratch.tile([P, 1024], mybir.dt.bfloat16,
                                          tag="ej", name=f"ej{k}_{et}")
                        nc.scalar.activation(
                            out=ej[:], in_=psa[:],
                            func=mybir.ActivationFunctionType.Exp,
                            scale=1.0 / LOGIT_SCALE,
                            accum_out=se_t[:, k * 4 + et: k * 4 + et + 1])
                        nc.vector.tensor_scalar(
                            dv[:, dvo_of[et]:dvo_of[et] + dw], psd[:, :dw],
                            K1, K2,
                            mybir.AluOpType.mult, mybir.AluOpType.add)
                        if last_pr:
                            # flush per unit, in completion order
                            nc.sync.dma_start(
                                out=dvs_d[:, k * dvtot + dvo_of[et]:
                                          k * dvtot + dvo_of[et] + dw],
                                in_=dv[:, dvo_of[et]:dvo_of[et] + dw])
                    if not last_pr:
                        # flush this half-chunk's DVE slice so the final
                        # DMA at kernel end is small
                        lo = dvo_of[2 * pr]
                        hi = dvo_of[2 * pr + 1] + dvws[2 * pr + 1]
                        nc.sync.dma_start(
                            out=dvs_d[:, k * dvtot + lo:k * dvtot + hi],
                            in_=dv[:, lo:hi])

            # ---- outputs ----------------------------------------------
            # se rides the (idle at the end) scalar queue, in parallel with
            # the last dv flushes on sync
            nc.scalar.dma_start(out=se_d[:], in_=se_t[:])

    nc.compile()
    return nc


def _prep_inputs(forward_embeds, backward_embeds, seq, fi, bi, w1, b1, w2, b2):
    import ml_dtypes
    fp8 = ml_dtypes.float8_e4m3

    fwd = np.asarray(forward_embeds, np.float32)
    bwd = np.asarray(backward_embeds, np.float32)
    seq = np.asarray(seq)
    fi = np.asarray(fi).astype(np.int64)
    bi = np.asarray(bi).astype(np.int64)
    w1 = np.asarray(w1, np.float32)
    b1 = np.asarray(b1, np.float32)
    w2 = np.asarray(w2, np.float32)
    b2 = np.asarray(b2, np.float32)

    B, L, Dd = fwd.shape
    assert Dd == D
    N = fi.shape[0]
    V = w2.shape[1] // 2
    R = B * N
    nch = (R + P - 1) // P
    rpad = nch * P
    vs = (2 * V) // NCORES

    assert not np.any(b2), "kernel assumes b2 == 0 (as in setup_inputs)"

    def q8(x):
        return np.clip(x, -240.0, 240.0).astype(fp8)

    # host-side gather + transpose (the sharding/layout prep)
    fb = np.concatenate([fwd[:, fi, :], bwd[:, bi, :]], axis=-1)  # [B, N, 2D]
    fb = fb.reshape(R, E)
    fbT = np.zeros((E, rpad), dtype=fp8)
    fbT[:, :R] = q8(fb.T * FB_SCALE)

    labels_f = seq[np.arange(B)[:, None], fi[None, :]].reshape(R).astype(np.int64)
    labels_b = seq[np.arange(B)[:, None], bi[None, :]].reshape(R).astype(np.int64)

    w1q = q8(w1 * W1_SCALE)
    b1s = (b1 * H_SCALE).astype(np.float32)

    shared = dict(fbt=fbT, w1=w1q, b1=b1s)
    in_maps = []
    for c in range(NCORES):
        m = dict(shared)
        m["w2s"] = q8(np.ascontiguousarray(w2[:, c * vs:(c + 1) * vs]) * W2_SCALE)
        in_maps.append(m)

    meta = dict(B=B, N=N, V=V, R=R, nch=nch, rpad=rpad, vs=vs,
                labels_f=labels_f, labels_b=labels_b, w2=w2)
    return in_maps, meta


def _combine(results, meta):
    import ml_dtypes
    R, nch, V = meta["R"], meta["nch"], meta["V"]
    # per-core partial sums of exp(logit) over its vocab slice
    S = []
    for c in range(NCORES):
        se = np.asarray(results[c]["se"], np.float64)          # [128, nch*4]
        s = se.reshape(P, nch, 4).sum(-1)                      # [128, nch]
        dvs = np.asarray(results[c]["dvs"])                    # [128, nch*3904]
        ex = dvs.view(ml_dtypes.bfloat16).astype(np.float32)
        s = s + ex.reshape(P, nch, -1).sum(-1)
        S.append(s.T.reshape(-1)[:R])                          # row-major [R]
    Sf = S[0] + S[1] + S[2] + S[3]
    Sb = S[4] + S[5] + S[6] + S[7]

    # label logits: host-side dot products against the exact (f32) w2
    # columns, using the device's fp8 h (ht = 16*h as fp8)
    ht = np.asarray(results[0]["ht"]).astype(np.float32)       # [128, 4, rpad]
    hv = ht.transpose(2, 1, 0).reshape(-1, D)[:R]              # [R, 512] = 16*h
    w2 = meta["w2"]
    labf = np.einsum('rd,dr->r', hv, w2[:, meta["labels_f"]],
                     dtype=np.float64) / H_SCALE
    labb = np.einsum('rd,dr->r', hv, w2[:, V + meta["labels_b"]],
                     dtype=np.float64) / H_SCALE

    nll_f = np.log(Sf) - labf
    nll_b = np.log(Sb) - labb
    loss = (1.0 * nll_f + 0.25 * nll_b).sum() / (R * 2)
    return np.float32(loss)


def kernel(**inputs) -> np.ndarray:
    in_maps, meta = _prep_inputs(**inputs)

    key = (meta["rpad"], meta["vs"])
    if key not in _nc_cache:
        _nc_cache[key] = build_program(*key)
    nc = _nc_cache[key]

    res = bass_utils.run_bass_kernel_spmd(nc, in_maps, core_ids=list(range(NCORES)))
    return _combine(res.results, meta)


if __name__ == "__main__":
    import reference
    ins = reference.setup_inputs()
    expected = np.asarray(reference.reference(**ins))
    actual = kernel(**{k: np.asarray(v) for k, v in ins.items()})
    rel = abs(float(actual) - float(expected)) / max(abs(float(expected)), 1e-9)
    print(f"expected {float(expected):.6f}  actual {float(actual):.6f}  rel {rel:.3e}")
